# revision 2
# baseline (speedup 1.0000x reference)
"""BERT self-attention (B=8, S=1024, D=1024, H=16, Dh=64) on 8 NeuronCores.

Sharding: pure data parallel — core b handles batch element b (B == n_cores),
qkv_weight replicated. No collectives.

Per-core dataflow (all matmuls bf16 with fp32 PSUM accumulation):
  1. X [S,D] loaded, cast to bf16 (GPSIMD), PE-transposed into X^T [D,S].
  2. W_v loaded+cast up front as [128, kt, 1024]; V computed with stationary
     X^T chunks and 512-wide moving W_v slices (128 matmuls total instead of
     512), laid out as V' [S, H*(Dh+1)] where each head's 65th column carries
     exp(mask): softmax(s + m) == exp(s)*exp(m) normalized, so the additive
     mask is an exact per-key row scaling of V', and the extra column makes
     the PV matmul emit softmax denominators for free.
  3. Per head pair: W_q/W_k column slices loaded (overlap with compute),
     Q^T,K^T computed as [features, S].
  4. Per head: scores^T [S_k,S_q] = (K^T chunk).T @ Q^T;  ACT computes
     exp(0.125*s) PSUM->SBUF(bf16);  ctx'^T [65,S_q] = V'.T @ expS^T;
     copied to SBUF bf16, PE-transposed (bf16, 1 cyc/row) back to [S_q,65],
     cols 0..63 multiplied by 1/col64.
  5. ctx assembled [S, D] fp32, DMA'd out.

PE instruction order interleaves per-pair QKV projections with the previous
heads' score/PV work so the tensor engine never waits on ACT exp results.

No max-subtraction in softmax: scores*scale is bounded (|x| <~ 4 for this
problem's scale) and exp runs in fp32 on ACT.
"""

import sys

import numpy as np

_REPO = "/opt/trn_rl_repo"
if _REPO not in sys.path:
    sys.path.insert(0, _REPO)

B, S, D, H, DH = 8, 1024, 1024, 16, 64
P = 128
NS = S // P          # seq tiles
NK = D // P          # contraction tiles
NHP = H // 2         # head pairs
NQ = 2               # 512-wide S_q chunks
QC = S // NQ         # 512
SCALE = 1.0 / 8.0    # 1/sqrt(DH)
VW = DH + 1          # V' width per head (extra denominator column)

_NC_CACHE = {}


def _build_nc():
    import concourse.bass as bass
    import concourse.tile as tile
    from concourse import bacc, mybir
    from concourse.masks import make_identity
    from contextlib import ExitStack

    f32 = mybir.dt.float32
    bf16 = mybir.dt.bfloat16
    Exp = mybir.ActivationFunctionType.Exp

    nc = bacc.Bacc("TRN2", target_bir_lowering=False, debug=False)
    x_d = nc.declare_dram_parameter("x", [S, D], f32, isOutput=False)
    w_d = nc.declare_dram_parameter("w", [D, 3 * D], f32, isOutput=False)
    m_d = nc.declare_dram_parameter("m", [S], f32, isOutput=False)
    o_d = nc.declare_dram_parameter("o", [S, D], f32, isOutput=True)

    with tile.TileContext(nc) as tc, ExitStack() as es:
        const = es.enter_context(tc.tile_pool(name="const", bufs=1))
        maskp = es.enter_context(tc.tile_pool(name="maskp", bufs=NS))
        xtp = es.enter_context(tc.tile_pool(name="xtp", bufs=NK))
        vp = es.enter_context(tc.tile_pool(name="vp", bufs=NS))
        ctxp = es.enter_context(tc.tile_pool(name="ctxp", bufs=NS))
        xstage = es.enter_context(tc.tile_pool(name="xstage", bufs=2))
        wvstage = es.enter_context(tc.tile_pool(name="wvstage", bufs=2))
        wvp = es.enter_context(tc.tile_pool(name="wvp", bufs=1))
        wstage = es.enter_context(tc.tile_pool(name="wstage", bufs=4))
        wqkp = es.enter_context(tc.tile_pool(name="wqkp", bufs=4))
        qktp = es.enter_context(tc.tile_pool(name="qktp", bufs=2))
        esp = es.enter_context(tc.tile_pool(name="esp", bufs=2 * NK))
        ctp = es.enter_context(tc.tile_pool(name="ctp", bufs=4))
        smallp = es.enter_context(tc.tile_pool(name="smallp", bufs=8))
        psA = es.enter_context(tc.tile_pool(name="psA", bufs=4, space="PSUM"))
        psC = es.enter_context(tc.tile_pool(name="psC", bufs=2, space="PSUM"))
        psT = es.enter_context(tc.tile_pool(name="psT", bufs=2, space="PSUM"))

        id_bf = const.tile([P, P], bf16, name="id_bf")
        make_identity(nc, id_bf)
        ones16 = const.tile([P, H], bf16, name="ones16")
        nc.vector.memset(ones16, 1.0)

        # mask -> exp(mask) per seq tile, [128,1] per-partition scalars
        em = []
        for st in range(NS):
            msk = maskp.tile([P, 1], f32, name=f"msk{st}", tag="msk")
            nc.sync.dma_start(
                out=msk,
                in_=m_d[st * P:(st + 1) * P].rearrange("(p o) -> p o", o=1),
            )
            emt = maskp.tile([P, 1], f32, name=f"em{st}", tag="em")
            nc.scalar.activation(emt, msk, Exp)
            em.append(emt)

        # persistent tensors
        xt = [xtp.tile([P, S], bf16, name=f"xt{j}", tag="xt") for j in range(NK)]
        v_sb = [vp.tile([P, H * VW], bf16, name=f"v{st}", tag="v") for st in range(NS)]
        ctx_sb = [ctxp.tile([P, D], f32, name=f"cx{st}", tag="cx") for st in range(NS)]

        # V' denominator columns = exp(mask) per key row
        for st in range(NS):
            vcols = v_sb[st].rearrange("p (h c) -> p h c", h=H)[:, :, DH]
            nc.scalar.mul(vcols, ones16, em[st])

        # X load, cast to bf16 (GPSIMD), PE-transpose into X^T
        for i in range(NS):
            xf = xstage.tile([P, D], f32, name=f"xf{i}", tag="xf")
            nc.sync.dma_start(out=xf, in_=x_d[i * P:(i + 1) * P, :])
            xb = xstage.tile([P, D], bf16, name=f"xb{i}", tag="xb")
            nc.gpsimd.tensor_copy(xb, xf)
            for j in range(NK):
                pst = psT.tile([P, P], bf16, name=f"px{i}_{j}", tag="pst")
                nc.tensor.transpose(pst, xb[:, j * P:(j + 1) * P], id_bf)
                nc.vector.tensor_copy(xt[j][:, i * P:(i + 1) * P], pst)

        # W_v full load + cast: [128, kt, 1024] bf16
        wvb = wvp.tile([P, NK, D], bf16, name="wvb")
        for q in range(4):
            wvf = wvstage.tile([P, 2, D], f32, name=f"wvf{q}", tag="wvf")
            nc.sync.dma_start(
                out=wvf,
                in_=w_d[2 * q * P:(2 * q + 2) * P, 2 * D:3 * D].rearrange(
                    "(kt p) c -> p kt c", p=P
                ),
            )
            nc.gpsimd.tensor_copy(wvb[:, 2 * q:2 * q + 2, :], wvf)

        def emit_v():
            # V' [S, H*(Dh+1)]: stationary X^T chunks, 512-wide W_v moving
            for st in range(NS):
                for half in range(2):
                    ps = psA.tile([P, QC], f32, name=f"pv{st}_{half}", tag="psA")
                    for k in range(NK):
                        nc.tensor.matmul(
                            ps,
                            xt[k][:, st * P:(st + 1) * P],
                            wvb[:, k, half * QC:(half + 1) * QC],
                            start=(k == 0),
                            stop=(k == NK - 1),
                        )
                    vdst = v_sb[st].rearrange("p (h c) -> p h c", h=H)[
                        :, half * 8:(half + 1) * 8, 0:DH
                    ]
                    vsrc = ps.rearrange("p (h c) -> p h c", h=8)
                    nc.scalar.mul(vdst, vsrc, em[st])

        def emit_qk(hp):
            # W_q/W_k column slices for this head pair, Q^T,K^T [128 feats, S]
            wbf = []
            for t, base in enumerate((hp * P, D + hp * P)):
                wf = wstage.tile([P, NK, P], f32, name=f"wf{hp}_{t}", tag="wf")
                nc.sync.dma_start(
                    out=wf,
                    in_=w_d[:, base:base + P].rearrange("(kt p) c -> p kt c", p=P),
                )
                wb = wqkp.tile([P, NK, P], bf16, name=f"wb{hp}_{t}", tag="wb")
                nc.gpsimd.tensor_copy(wb, wf)
                wbf.append(wb)
            qt_t = qktp.tile([P, S], bf16, name=f"qt{hp}", tag="qt")
            kt_t = qktp.tile([P, S], bf16, name=f"kt{hp}", tag="kt")
            for wsel, dest in ((0, qt_t), (1, kt_t)):
                for n in range(NQ):
                    ps = psA.tile([P, QC], f32, name=f"pq{hp}_{wsel}_{n}", tag="psA")
                    for k in range(NK):
                        nc.tensor.matmul(
                            ps,
                            wbf[wsel][:, k, :],
                            xt[k][:, n * QC:(n + 1) * QC],
                            start=(k == 0),
                            stop=(k == NK - 1),
                        )
                    nc.vector.tensor_copy(dest[:, n * QC:(n + 1) * QC], ps)
            return qt_t, kt_t

        def emit_scores(h, qt_t, kt_t):
            h2 = h % 2
            hs = h2 * DH
            esb = [
                esp.tile([P, S], bf16, name=f"e{h}_{k}", tag="es") for k in range(NK)
            ]
            for k in range(NK):
                for qn in range(NQ):
                    ps = psA.tile([P, QC], f32, name=f"s{h}_{k}_{qn}", tag="psA")
                    nc.tensor.matmul(
                        ps,
                        kt_t[hs:hs + DH, k * P:(k + 1) * P],
                        qt_t[hs:hs + DH, qn * QC:(qn + 1) * QC],
                        start=True,
                        stop=True,
                    )
                    nc.scalar.activation(
                        esb[k][:, qn * QC:(qn + 1) * QC], ps, Exp, scale=SCALE
                    )
            return esb

        def emit_pv(h, esb):
            # ctx'^T [65, S_q] = V'.T @ expS^T; bf16 transpose back + normalize
            for qn in range(NQ):
                psc = psC.tile([VW, QC], f32, name=f"c{h}_{qn}", tag="psC")
                for k in range(NK):
                    nc.tensor.matmul(
                        psc,
                        v_sb[k][:, h * VW:(h + 1) * VW],
                        esb[k][:, qn * QC:(qn + 1) * QC],
                        start=(k == 0),
                        stop=(k == NK - 1),
                    )
                ct = ctp.tile([VW, QC], bf16, name=f"ct{h}_{qn}", tag="ct")
                nc.vector.tensor_copy(ct, psc)
                for qs in range(QC // P):
                    qi = qn * (QC // P) + qs
                    pst = psT.tile([P, P], bf16, name=f"pt{h}_{qi}", tag="pst")
                    nc.tensor.transpose(
                        pst[:, 0:VW], ct[:, qs * P:(qs + 1) * P], id_bf[0:VW, 0:VW]
                    )
                    rec = smallp.tile([P, 1], f32, name=f"r{h}_{qi}", tag="rec")
                    nc.vector.reciprocal(rec, pst[:, DH:DH + 1])
                    nc.scalar.mul(
                        ctx_sb[qi][:, h * DH:(h + 1) * DH], pst[:, 0:DH], rec
                    )

        # PE order: QK(0), scores(h0), scores(h1), V, PV(h0), PV(h1),
        # QK(1), scores(h2), PV(h2), scores(h3), PV(h3), QK(2), ...
        qt0, kt0 = emit_qk(0)
        es0 = emit_scores(0, qt0, kt0)
        es1 = emit_scores(1, qt0, kt0)
        emit_v()
        emit_pv(0, es0)
        emit_pv(1, es1)
        for hp in range(1, NHP):
            qt_t, kt_t = emit_qk(hp)
            for h2 in range(2):
                h = 2 * hp + h2
                esb = emit_scores(h, qt_t, kt_t)
                emit_pv(h, esb)

        for st in range(NS):
            nc.sync.dma_start(out=o_d[st * P:(st + 1) * P, :], in_=ctx_sb[st])

    nc.finalize()
    return nc


def _get_nc():
    if "nc" not in _NC_CACHE:
        _NC_CACHE["nc"] = _build_nc()
    return _NC_CACHE["nc"]


def _run(hidden_states, attention_mask, qkv_weight, trace=False, **trace_kw):
    from concourse.bass_utils import run_bass_kernel_spmd

    nc = _get_nc()
    hidden = np.ascontiguousarray(np.asarray(hidden_states, dtype=np.float32))
    mask = np.ascontiguousarray(
        np.asarray(attention_mask, dtype=np.float32).reshape(B, S)
    )
    w = np.ascontiguousarray(np.asarray(qkv_weight, dtype=np.float32))
    in_maps = [
        {"x": hidden[b], "w": w, "m": mask[b]} for b in range(B)
    ]
    res = run_bass_kernel_spmd(nc, in_maps, list(range(B)), trace=trace, **trace_kw)
    out = np.stack([np.asarray(res.results[b]["o"]) for b in range(B)], axis=0)
    return out.astype(np.float32), res


def kernel(hidden_states, attention_mask, qkv_weight):
    out, _ = _run(hidden_states, attention_mask, qkv_weight, trace=False)
    return out


# revision 3
# speedup vs baseline: 1.0823x; 1.0823x over previous
"""BERT self-attention (B=8, S=1024, D=1024, H=16, Dh=64) on 8 NeuronCores.

Sharding: pure data parallel — core b handles batch element b (B == n_cores),
qkv_weight replicated. No collectives.

Per-core dataflow (all matmuls bf16 with fp32 PSUM accumulation):
  1. X [S,D] loaded, cast to bf16 (DVE), PE-transposed into X^T [D,S].
  2. W_v loaded+cast up front as [128, kt, 1024]; V computed with stationary
     X^T chunks and 512-wide moving W_v slices (128 matmuls total instead of
     512), laid out as V' [S, H*(Dh+1)] where each head's 65th column carries
     exp(mask): softmax(s + m) == exp(s)*exp(m) normalized, so the additive
     mask is an exact per-key row scaling of V', and the extra column makes
     the PV matmul emit softmax denominators for free.
  3. Per head pair: W_q/W_k column slices loaded (overlap with compute),
     Q^T,K^T computed as [features, S].
  4. Per head: scores^T [S_k,S_q] = (K^T chunk).T @ Q^T;  ACT computes
     exp(0.125*s) PSUM->SBUF(bf16);  ctx'^T [65,S_q] = V'.T @ expS^T;
     copied to SBUF bf16, PE-transposed (bf16, 1 cyc/row) back to [S_q,65],
     cols 0..63 multiplied by 1/col64.
  5. ctx assembled [S, D] fp32, DMA'd out.

PE instruction order interleaves per-pair QKV projections with the previous
heads' score/PV work so the tensor engine never waits on ACT exp results.

No max-subtraction in softmax: scores*scale is bounded (|x| <~ 4 for this
problem's scale) and exp runs in fp32 on ACT.
"""

import sys

import numpy as np

_REPO = "/opt/trn_rl_repo"
if _REPO not in sys.path:
    sys.path.insert(0, _REPO)

B, S, D, H, DH = 8, 1024, 1024, 16, 64
P = 128
NS = S // P          # seq tiles
NK = D // P          # contraction tiles
NHP = H // 2         # head pairs
NQ = 2               # 512-wide S_q chunks
QC = S // NQ         # 512
SCALE = 1.0 / 8.0    # 1/sqrt(DH)
VW = DH + 1          # V' width per head (extra denominator column)

_NC_CACHE = {}


def _build_nc():
    import concourse.bass as bass
    import concourse.tile as tile
    from concourse import bacc, mybir
    from concourse.masks import make_identity
    from contextlib import ExitStack

    f32 = mybir.dt.float32
    bf16 = mybir.dt.bfloat16
    Exp = mybir.ActivationFunctionType.Exp

    nc = bacc.Bacc("TRN2", target_bir_lowering=False, debug=False)
    x_d = nc.declare_dram_parameter("x", [S, D], f32, isOutput=False)
    w_d = nc.declare_dram_parameter("w", [D, 3 * D], f32, isOutput=False)
    m_d = nc.declare_dram_parameter("m", [S], f32, isOutput=False)
    o_d = nc.declare_dram_parameter("o", [S, D], f32, isOutput=True)

    with tile.TileContext(nc) as tc, ExitStack() as es:
        const = es.enter_context(tc.tile_pool(name="const", bufs=1))
        maskp = es.enter_context(tc.tile_pool(name="maskp", bufs=NS))
        xtp = es.enter_context(tc.tile_pool(name="xtp", bufs=NK))
        vp = es.enter_context(tc.tile_pool(name="vp", bufs=NS))
        ctxp = es.enter_context(tc.tile_pool(name="ctxp", bufs=NS))
        xstage = es.enter_context(tc.tile_pool(name="xstage", bufs=2))
        wvstage = es.enter_context(tc.tile_pool(name="wvstage", bufs=2))
        wvp = es.enter_context(tc.tile_pool(name="wvp", bufs=1))
        wstage = es.enter_context(tc.tile_pool(name="wstage", bufs=4))
        wqkp = es.enter_context(tc.tile_pool(name="wqkp", bufs=4))
        qktp = es.enter_context(tc.tile_pool(name="qktp", bufs=2))
        esp = es.enter_context(tc.tile_pool(name="esp", bufs=2 * NK))
        ctp = es.enter_context(tc.tile_pool(name="ctp", bufs=4))
        smallp = es.enter_context(tc.tile_pool(name="smallp", bufs=8))
        psA = es.enter_context(tc.tile_pool(name="psA", bufs=4, space="PSUM"))
        psC = es.enter_context(tc.tile_pool(name="psC", bufs=2, space="PSUM"))
        psT = es.enter_context(tc.tile_pool(name="psT", bufs=2, space="PSUM"))

        id_bf = const.tile([P, P], bf16, name="id_bf")
        make_identity(nc, id_bf)
        ones16 = const.tile([P, H], bf16, name="ones16")
        nc.vector.memset(ones16, 1.0)

        # mask -> exp(mask) per seq tile, [128,1] per-partition scalars
        em = []
        for st in range(NS):
            msk = maskp.tile([P, 1], f32, name=f"msk{st}", tag="msk")
            nc.sync.dma_start(
                out=msk,
                in_=m_d[st * P:(st + 1) * P].rearrange("(p o) -> p o", o=1),
            )
            emt = maskp.tile([P, 1], f32, name=f"em{st}", tag="em")
            nc.scalar.activation(emt, msk, Exp)
            em.append(emt)

        # persistent tensors
        xt = [xtp.tile([P, S], bf16, name=f"xt{j}", tag="xt") for j in range(NK)]
        v_sb = [vp.tile([P, H * VW], bf16, name=f"v{st}", tag="v") for st in range(NS)]
        ctx_sb = [ctxp.tile([P, D], f32, name=f"cx{st}", tag="cx") for st in range(NS)]

        # V' denominator columns = exp(mask) per key row
        for st in range(NS):
            vcols = v_sb[st].rearrange("p (h c) -> p h c", h=H)[:, :, DH]
            nc.scalar.mul(vcols, ones16, em[st])

        # X load, cast to bf16 (DVE), PE-transpose into X^T
        for i in range(NS):
            xf = xstage.tile([P, D], f32, name=f"xf{i}", tag="xf")
            nc.sync.dma_start(out=xf, in_=x_d[i * P:(i + 1) * P, :])
            xb = xstage.tile([P, D], bf16, name=f"xb{i}", tag="xb")
            nc.vector.tensor_copy(xb, xf)
            for j in range(NK):
                pst = psT.tile([P, P], bf16, name=f"px{i}_{j}", tag="pst")
                nc.tensor.transpose(pst, xb[:, j * P:(j + 1) * P], id_bf)
                nc.vector.tensor_copy(xt[j][:, i * P:(i + 1) * P], pst)

        # W_v full load + cast: [128, kt, 1024] bf16
        wvb = wvp.tile([P, NK, D], bf16, name="wvb")
        for q in range(4):
            wvf = wvstage.tile([P, 2, D], f32, name=f"wvf{q}", tag="wvf")
            nc.sync.dma_start(
                out=wvf,
                in_=w_d[2 * q * P:(2 * q + 2) * P, 2 * D:3 * D].rearrange(
                    "(kt p) c -> p kt c", p=P
                ),
            )
            nc.vector.tensor_copy(wvb[:, 2 * q:2 * q + 2, :], wvf)

        def emit_v():
            # V' [S, H*(Dh+1)]: stationary X^T chunks, 512-wide W_v moving
            for st in range(NS):
                for half in range(2):
                    ps = psA.tile([P, QC], f32, name=f"pv{st}_{half}", tag="psA")
                    for k in range(NK):
                        nc.tensor.matmul(
                            ps,
                            xt[k][:, st * P:(st + 1) * P],
                            wvb[:, k, half * QC:(half + 1) * QC],
                            start=(k == 0),
                            stop=(k == NK - 1),
                        )
                    vdst = v_sb[st].rearrange("p (h c) -> p h c", h=H)[
                        :, half * 8:(half + 1) * 8, 0:DH
                    ]
                    vsrc = ps.rearrange("p (h c) -> p h c", h=8)
                    nc.scalar.mul(vdst, vsrc, em[st])

        def emit_qk(hp):
            # W_q/W_k column slices for this head pair, Q^T,K^T [128 feats, S]
            wbf = []
            for t, base in enumerate((hp * P, D + hp * P)):
                wf = wstage.tile([P, NK, P], f32, name=f"wf{hp}_{t}", tag="wf")
                nc.sync.dma_start(
                    out=wf,
                    in_=w_d[:, base:base + P].rearrange("(kt p) c -> p kt c", p=P),
                )
                wb = wqkp.tile([P, NK, P], bf16, name=f"wb{hp}_{t}", tag="wb")
                nc.vector.tensor_copy(wb, wf)
                wbf.append(wb)
            qt_t = qktp.tile([P, S], bf16, name=f"qt{hp}", tag="qt")
            kt_t = qktp.tile([P, S], bf16, name=f"kt{hp}", tag="kt")
            for wsel, dest in ((0, qt_t), (1, kt_t)):
                for n in range(NQ):
                    ps = psA.tile([P, QC], f32, name=f"pq{hp}_{wsel}_{n}", tag="psA")
                    for k in range(NK):
                        nc.tensor.matmul(
                            ps,
                            wbf[wsel][:, k, :],
                            xt[k][:, n * QC:(n + 1) * QC],
                            start=(k == 0),
                            stop=(k == NK - 1),
                        )
                    nc.vector.tensor_copy(dest[:, n * QC:(n + 1) * QC], ps)
            return qt_t, kt_t

        def emit_scores(h, qt_t, kt_t):
            h2 = h % 2
            hs = h2 * DH
            esb = [
                esp.tile([P, S], bf16, name=f"e{h}_{k}", tag="es") for k in range(NK)
            ]
            for k in range(NK):
                for qn in range(NQ):
                    ps = psA.tile([P, QC], f32, name=f"s{h}_{k}_{qn}", tag="psA")
                    nc.tensor.matmul(
                        ps,
                        kt_t[hs:hs + DH, k * P:(k + 1) * P],
                        qt_t[hs:hs + DH, qn * QC:(qn + 1) * QC],
                        start=True,
                        stop=True,
                    )
                    nc.scalar.activation(
                        esb[k][:, qn * QC:(qn + 1) * QC], ps, Exp, scale=SCALE
                    )
            return esb

        def emit_pv(h, esb):
            # ctx'^T [65, S_q] = V'.T @ expS^T; bf16 transpose back + normalize
            for qn in range(NQ):
                psc = psC.tile([VW, QC], f32, name=f"c{h}_{qn}", tag="psC")
                for k in range(NK):
                    nc.tensor.matmul(
                        psc,
                        v_sb[k][:, h * VW:(h + 1) * VW],
                        esb[k][:, qn * QC:(qn + 1) * QC],
                        start=(k == 0),
                        stop=(k == NK - 1),
                    )
                ct = ctp.tile([VW, QC], bf16, name=f"ct{h}_{qn}", tag="ct")
                nc.vector.tensor_copy(ct, psc)
                for qs in range(QC // P):
                    qi = qn * (QC // P) + qs
                    pst = psT.tile([P, P], bf16, name=f"pt{h}_{qi}", tag="pst")
                    nc.tensor.transpose(
                        pst[:, 0:VW], ct[:, qs * P:(qs + 1) * P], id_bf[0:VW, 0:VW]
                    )
                    rec = smallp.tile([P, 1], f32, name=f"r{h}_{qi}", tag="rec")
                    nc.vector.reciprocal(rec, pst[:, DH:DH + 1])
                    nc.scalar.mul(
                        ctx_sb[qi][:, h * DH:(h + 1) * DH], pst[:, 0:DH], rec
                    )

        # PE order: QK(0), scores(h0), scores(h1), V, PV(h0), PV(h1),
        # QK(1), scores(h2), PV(h2), scores(h3), PV(h3), QK(2), ...
        qt0, kt0 = emit_qk(0)
        es0 = emit_scores(0, qt0, kt0)
        es1 = emit_scores(1, qt0, kt0)
        emit_v()
        emit_pv(0, es0)
        emit_pv(1, es1)
        for hp in range(1, NHP):
            qt_t, kt_t = emit_qk(hp)
            for h2 in range(2):
                h = 2 * hp + h2
                esb = emit_scores(h, qt_t, kt_t)
                emit_pv(h, esb)

        for st in range(NS):
            nc.sync.dma_start(out=o_d[st * P:(st + 1) * P, :], in_=ctx_sb[st])

    nc.finalize()
    return nc


def _get_nc():
    if "nc" not in _NC_CACHE:
        _NC_CACHE["nc"] = _build_nc()
    return _NC_CACHE["nc"]


def _run(hidden_states, attention_mask, qkv_weight, trace=False, **trace_kw):
    from concourse.bass_utils import run_bass_kernel_spmd

    nc = _get_nc()
    hidden = np.ascontiguousarray(np.asarray(hidden_states, dtype=np.float32))
    mask = np.ascontiguousarray(
        np.asarray(attention_mask, dtype=np.float32).reshape(B, S)
    )
    w = np.ascontiguousarray(np.asarray(qkv_weight, dtype=np.float32))
    in_maps = [
        {"x": hidden[b], "w": w, "m": mask[b]} for b in range(B)
    ]
    res = run_bass_kernel_spmd(nc, in_maps, list(range(B)), trace=trace, **trace_kw)
    out = np.stack([np.asarray(res.results[b]["o"]) for b in range(B)], axis=0)
    return out.astype(np.float32), res


def kernel(hidden_states, attention_mask, qkv_weight):
    out, _ = _run(hidden_states, attention_mask, qkv_weight, trace=False)
    return out


# revision 7
# speedup vs baseline: 1.1978x; 1.1067x over previous
"""BERT self-attention (B=8, S=1024, D=1024, H=16, Dh=64) on 8 NeuronCores.

Sharding: pure data parallel — core b handles batch element b (B == n_cores),
qkv_weight replicated. No collectives.

Per-core dataflow (all matmuls bf16 with fp32 PSUM accumulation):
  1. X [S,D] loaded (prefetched 4 deep), cast to bf16 (DVE), PE-transposed
     into X^T [D,S] in groups of 4 chunks per PSUM unload; unloads alternate
     between DVE and ACT (idle early).
  2. W_v loaded+cast up front as [128, kt, 1024]; V computed with stationary
     X^T chunks and 512-wide moving W_v slices (128 matmuls), laid out as
     V' [S, H*(Dh+1)] where each head's 65th column carries exp(mask):
     softmax(s + m) == exp(s)*exp(m) normalized, so the additive mask is an
     exact per-key row scaling of V', and the extra column makes the PV
     matmul emit softmax denominators for free.
  3. Per head pair: W_q/W_k column slices loaded one pair ahead,
     Q^T,K^T computed as [features, S].
  4. Per head: scores^T [S_k,S_q] = (K^T chunk).T @ Q^T;  ACT computes
     exp(0.125*s) PSUM->SBUF(bf16);  ctx'^T [65,S_q] = V'.T @ expS^T;
     copied to SBUF bf16, PE-transposed (bf16) back to [S_q,65] four chunks
     per PSUM tile, one strided reciprocal per 4 denominators, cols 0..63
     multiplied by 1/col64 on ACT.
  5. ctx assembled [S, D] fp32, DMA'd out in 4 column groups as head
     quartets complete, hiding the output transfer behind compute.

PE emission order pipelines stages so the tensor engine never waits on
ACT/DVE results: each head's ctx transposes are deferred behind the next
score block, and PV(h) runs after the next pair's QKV projection.

No max-subtraction in softmax: scores*scale is bounded (|x| <~ 4 for this
problem's scale) and exp runs in fp32 on ACT.
"""

import sys

import numpy as np

_REPO = "/opt/trn_rl_repo"
if _REPO not in sys.path:
    sys.path.insert(0, _REPO)

B, S, D, H, DH = 8, 1024, 1024, 16, 64
P = 128
NS = S // P          # seq tiles
NK = D // P          # contraction tiles
NHP = H // 2         # head pairs
NQ = 2               # 512-wide S_q chunks
QC = S // NQ         # 512
SCALE = 1.0 / 8.0    # 1/sqrt(DH)
VW = DH + 1          # V' width per head (extra denominator column)

_NC_CACHE = {}


def _build_nc():
    import concourse.bass as bass
    import concourse.tile as tile
    from concourse import bacc, mybir
    from concourse.masks import make_identity
    from contextlib import ExitStack

    f32 = mybir.dt.float32
    bf16 = mybir.dt.bfloat16
    Exp = mybir.ActivationFunctionType.Exp

    nc = bacc.Bacc("TRN2", target_bir_lowering=False, debug=False)
    x_d = nc.declare_dram_parameter("x", [S, D], f32, isOutput=False)
    w_d = nc.declare_dram_parameter("w", [D, 3 * D], f32, isOutput=False)
    m_d = nc.declare_dram_parameter("m", [S], f32, isOutput=False)
    o_d = nc.declare_dram_parameter("o", [S, D], f32, isOutput=True)

    with tile.TileContext(nc) as tc, ExitStack() as es:
        const = es.enter_context(tc.tile_pool(name="const", bufs=1))
        maskp = es.enter_context(tc.tile_pool(name="maskp", bufs=NS))
        xtp = es.enter_context(tc.tile_pool(name="xtp", bufs=1))
        vp = es.enter_context(tc.tile_pool(name="vp", bufs=NS))
        ctxp = es.enter_context(tc.tile_pool(name="ctxp", bufs=NS))
        xstage = es.enter_context(tc.tile_pool(name="xstage", bufs=2))
        wvstage = es.enter_context(tc.tile_pool(name="wvstage", bufs=2))
        wvp = es.enter_context(tc.tile_pool(name="wvp", bufs=1))
        wstage = es.enter_context(tc.tile_pool(name="wstage", bufs=4))
        wqkp = es.enter_context(tc.tile_pool(name="wqkp", bufs=4))
        qktp = es.enter_context(tc.tile_pool(name="qktp", bufs=2))
        esp = es.enter_context(tc.tile_pool(name="esp", bufs=2 * NK))
        ctp = es.enter_context(tc.tile_pool(name="ctp", bufs=4))
        smallp = es.enter_context(tc.tile_pool(name="smallp", bufs=8))
        psA = es.enter_context(tc.tile_pool(name="psA", bufs=4, space="PSUM"))
        psC = es.enter_context(tc.tile_pool(name="psC", bufs=2, space="PSUM"))
        psT = es.enter_context(tc.tile_pool(name="psT", bufs=2, space="PSUM"))

        id_bf = const.tile([P, P], bf16, name="id_bf")
        make_identity(nc, id_bf)
        ones16 = const.tile([P, H], bf16, name="ones16")
        nc.vector.memset(ones16, 1.0)

        # mask -> exp(mask) per seq tile, [128,1] per-partition scalars
        em = []
        for st in range(NS):
            msk = maskp.tile([P, 1], f32, name=f"msk{st}", tag="msk")
            nc.sync.dma_start(
                out=msk,
                in_=m_d[st * P:(st + 1) * P].rearrange("(p o) -> p o", o=1),
            )
            emt = maskp.tile([P, 1], f32, name=f"em{st}", tag="em")
            nc.scalar.activation(emt, msk, Exp)
            em.append(emt)

        # persistent tensors
        xt = xtp.tile([P, NK, S], bf16, name="xt")  # X^T: [d-part, kt, s]
        v_sb = [vp.tile([P, H * VW], bf16, name=f"v{st}", tag="v") for st in range(NS)]
        ctx_sb = [ctxp.tile([P, D], f32, name=f"cx{st}", tag="cx") for st in range(NS)]

        # V' denominator columns = exp(mask) per key row
        for st in range(NS):
            vcols = v_sb[st].rearrange("p (h c) -> p h c", h=H)[:, :, DH]
            nc.scalar.mul(vcols, ones16, em[st])

        def qk_load(hp):
            # W_q/W_k column slices for this head pair: DMA + bf16 cast (DVE)
            wbf = []
            for t, base in enumerate((hp * P, D + hp * P)):
                wf = wstage.tile([P, NK, P], f32, name=f"wf{hp}_{t}", tag="wf")
                nc.sync.dma_start(
                    out=wf,
                    in_=w_d[:, base:base + P].rearrange("(kt p) c -> p kt c", p=P),
                )
                wb = wqkp.tile([P, NK, P], bf16, name=f"wb{hp}_{t}", tag="wb")
                nc.vector.tensor_copy(wb, wf)
                wbf.append(wb)
            return wbf

        # pair-0 W slices first: small DMA ahead of the X / W_v bulk
        wbf0 = qk_load(0)

        # X load (prefetched 4 deep)
        xfs = []
        for i in range(NS):
            xf = xstage.tile([P, D], f32, name=f"xf{i}", tag="xf", bufs=4)
            nc.sync.dma_start(out=xf, in_=x_d[i * P:(i + 1) * P, :])
            xfs.append(xf)

        # W_v full load: [128, kt, 1024]
        wvb = wvp.tile([P, NK, D], bf16, name="wvb")
        wvfs = []
        for q in range(4):
            wvf = wvstage.tile([P, 2, D], f32, name=f"wvf{q}", tag="wvf")
            nc.sync.dma_start(
                out=wvf,
                in_=w_d[2 * q * P:(2 * q + 2) * P, 2 * D:3 * D].rearrange(
                    "(kt p) c -> p kt c", p=P
                ),
            )
            wvfs.append(wvf)

        # X cast + PE transpose; PSUM unloads in groups of 4 chunks,
        # alternating DVE / ACT
        for i in range(NS):
            xb = xstage.tile([P, D], bf16, name=f"xb{i}", tag="xb", bufs=2)
            nc.vector.tensor_copy(xb, xfs[i])
            for g in range(2):
                pst = psT.tile([P, 4 * P], bf16, name=f"px{i}_{g}", tag="pst")
                for c in range(4):
                    j = 4 * g + c
                    nc.tensor.transpose(
                        pst[:, c * P:(c + 1) * P], xb[:, j * P:(j + 1) * P], id_bf
                    )
                eng = nc.vector if (2 * i + g) % 2 else nc.scalar
                dst = xt[:, 4 * g:4 * g + 4, i * P:(i + 1) * P]
                src = pst.rearrange("p (c q) -> p c q", c=4)
                if eng is nc.vector:
                    eng.tensor_copy(dst, src)
                else:
                    eng.copy(dst, src)

        def qk_compute(hp, wbf):
            qt_t = qktp.tile([P, S], bf16, name=f"qt{hp}", tag="qt")
            kt_t = qktp.tile([P, S], bf16, name=f"kt{hp}", tag="kt")
            for wsel, dest in ((1, kt_t), (0, qt_t)):
                for n in range(NQ):
                    ps = psA.tile([P, QC], f32, name=f"pq{hp}_{wsel}_{n}", tag="psA")
                    for k in range(NK):
                        nc.tensor.matmul(
                            ps,
                            wbf[wsel][:, k, :],
                            xt[:, k, n * QC:(n + 1) * QC],
                            start=(k == 0),
                            stop=(k == NK - 1),
                        )
                    nc.vector.tensor_copy(dest[:, n * QC:(n + 1) * QC], ps)
            return qt_t, kt_t

        def emit_v():
            # V' [S, H*(Dh+1)]: stationary X^T chunks, 512-wide W_v moving
            for st in range(NS):
                for half in range(2):
                    ps = psA.tile([P, QC], f32, name=f"pv{st}_{half}", tag="psA")
                    for k in range(NK):
                        nc.tensor.matmul(
                            ps,
                            xt[:, k, st * P:(st + 1) * P],
                            wvb[:, k, half * QC:(half + 1) * QC],
                            start=(k == 0),
                            stop=(k == NK - 1),
                        )
                    vdst = v_sb[st].rearrange("p (h c) -> p h c", h=H)[
                        :, half * 8:(half + 1) * 8, 0:DH
                    ]
                    vsrc = ps.rearrange("p (h c) -> p h c", h=8)
                    nc.scalar.mul(vdst, vsrc, em[st])

        def emit_scores(h, qt_t, kt_t):
            hs = (h % 2) * DH
            esb = [
                esp.tile([P, S], bf16, name=f"e{h}_{k}", tag="es") for k in range(NK)
            ]
            for k in range(NK):
                for qn in range(NQ):
                    ps = psA.tile([P, QC], f32, name=f"s{h}_{k}_{qn}", tag="psA")
                    nc.tensor.matmul(
                        ps,
                        kt_t[hs:hs + DH, k * P:(k + 1) * P],
                        qt_t[hs:hs + DH, qn * QC:(qn + 1) * QC],
                        start=True,
                        stop=True,
                    )
                    nc.scalar.activation(
                        esb[k][:, qn * QC:(qn + 1) * QC], ps, Exp, scale=SCALE
                    )
            return esb

        def emit_pv(h, esb):
            # ctx'^T [65, S_q] = V'.T @ expS^T; SBUF bf16 copy (DVE)
            cts = []
            for qn in range(NQ):
                psc = psC.tile([VW, QC], f32, name=f"c{h}_{qn}", tag="psC")
                for k in range(NK):
                    nc.tensor.matmul(
                        psc,
                        v_sb[k][:, h * VW:(h + 1) * VW],
                        esb[k][:, qn * QC:(qn + 1) * QC],
                        start=(k == 0),
                        stop=(k == NK - 1),
                    )
                ct = ctp.tile([VW, QC], bf16, name=f"ct{h}_{qn}", tag="ct")
                nc.vector.tensor_copy(ct, psc)
                cts.append(ct)
            return cts

        def emit_ctxt(h, cts):
            # 4 bf16 PE transposes per PSUM tile back to [S_q, 65];
            # one strided reciprocal per 4 denominators; normalize on ACT
            VW2 = VW + 1  # 66: keeps each chunk's PSUM byte offset 4B-aligned
            for qn in range(NQ):
                ct = cts[qn]
                pst = psT.tile([P, 4 * VW2], bf16, name=f"pt{h}_{qn}", tag="pst")
                for qs in range(QC // P):
                    nc.tensor.transpose(
                        pst[:, qs * VW2:qs * VW2 + VW],
                        ct[:, qs * P:(qs + 1) * P],
                        id_bf[0:VW, 0:VW],
                    )
                rec = smallp.tile([P, 4], f32, name=f"r{h}_{qn}", tag="rec")
                pst4 = pst.rearrange("p (c w) -> p c w", w=VW2)
                nc.vector.reciprocal(rec, pst4[:, 0:4, DH])
                for qs in range(QC // P):
                    qi = qn * (QC // P) + qs
                    nc.scalar.mul(
                        ctx_sb[qi][:, h * DH:(h + 1) * DH],
                        pst[:, qs * VW2:qs * VW2 + DH],
                        rec[:, qs:qs + 1],
                    )

        def emit_out_quarter(qtr):
            # heads 4*qtr..4*qtr+3 done for every row: columns are final
            c0 = qtr * 4 * DH
            for st in range(NS):
                nc.sync.dma_start(
                    out=o_d[st * P:(st + 1) * P, c0:c0 + 4 * DH],
                    in_=ctx_sb[st][:, c0:c0 + 4 * DH],
                )

        # PE order: X^T, QK0, sc0, sc1, V, PV0, then per pair p>=1:
        #   QK(p), PV(2p-1), ctxT(2p-2), sc(2p), ctxT(2p-1), sc(2p+1), PV(2p)
        # with ctx transposes riding one stage behind their PV, so the PE
        # never waits on the DVE ct copies.
        qt0, kt0 = qk_compute(0, wbf0)
        # W_v casts deferred here: DVE reaches them after the QK0 unloads
        for q in range(4):
            nc.vector.tensor_copy(wvb[:, 2 * q:2 * q + 2, :], wvfs[q])
        es = {0: emit_scores(0, qt0, kt0), 1: emit_scores(1, qt0, kt0)}
        emit_v()
        cts = {0: emit_pv(0, es.pop(0))}
        for hp in range(1, NHP):
            wbf = qk_load(hp)
            qt_t, kt_t = qk_compute(hp, wbf)
            cts[2 * hp - 1] = emit_pv(2 * hp - 1, es.pop(2 * hp - 1))
            emit_ctxt(2 * hp - 2, cts.pop(2 * hp - 2))
            es[2 * hp] = emit_scores(2 * hp, qt_t, kt_t)
            emit_ctxt(2 * hp - 1, cts.pop(2 * hp - 1))
            es[2 * hp + 1] = emit_scores(2 * hp + 1, qt_t, kt_t)
            cts[2 * hp] = emit_pv(2 * hp, es.pop(2 * hp))
            if hp == 3:
                emit_out_quarter(0)
            elif hp == 5:
                emit_out_quarter(1)
            elif hp == 7:
                emit_out_quarter(2)
        cts[15] = emit_pv(15, es.pop(15))
        emit_ctxt(14, cts.pop(14))
        emit_ctxt(15, cts.pop(15))
        emit_out_quarter(3)

    nc.finalize()
    return nc


def _get_nc():
    if "nc" not in _NC_CACHE:
        _NC_CACHE["nc"] = _build_nc()
    return _NC_CACHE["nc"]


def _run(hidden_states, attention_mask, qkv_weight, trace=False, **trace_kw):
    from concourse.bass_utils import run_bass_kernel_spmd

    nc = _get_nc()
    hidden = np.ascontiguousarray(np.asarray(hidden_states, dtype=np.float32))
    mask = np.ascontiguousarray(
        np.asarray(attention_mask, dtype=np.float32).reshape(B, S)
    )
    w = np.ascontiguousarray(np.asarray(qkv_weight, dtype=np.float32))
    in_maps = [
        {"x": hidden[b], "w": w, "m": mask[b]} for b in range(B)
    ]
    res = run_bass_kernel_spmd(nc, in_maps, list(range(B)), trace=trace, **trace_kw)
    out = np.stack([np.asarray(res.results[b]["o"]) for b in range(B)], axis=0)
    return out.astype(np.float32), res


def kernel(hidden_states, attention_mask, qkv_weight):
    out, _ = _run(hidden_states, attention_mask, qkv_weight, trace=False)
    return out


if __name__ == "__main__":
    # quick shape smoke test via the interpreter-free build
    _build_nc()
    print("build ok")
